# revision 9
# baseline (speedup 1.0000x reference)
"""Weighted-BCE loss kernel for Trainium2 (8 NeuronCores, SPMD data-parallel).

Reference math (torch-style BCELoss with class-balancing weights):
    n   = len(x), s = sum(gt)
    w0  = n / (2*(n-s)),  w1 = n / (2*s)
    L1  = max(log(x),     -100)
    L0  = max(log1p(-x),  -100)
    loss = mean( where(gt==0, w0, w1) * -(gt*L1 + (1-gt)*L0) )

The weights depend only on the GLOBAL positive count s, so the loss
decomposes into 4 global sums computed shard-locally:
    A = sum(gt * L1),  B = sum(gt * L0),  C = sum(L0),  s = sum(gt)
    loss = -( A/(2s) + (C-B)/(2(n-s)) )

Each core processes a 1/8 shard laid out [128 partitions, 16384 free]:
  - ScalarE (ACT): Ln(x), and Ln(1-x) via the free affine (scale=-1,
    bias=1); the second op's accum_out produces C for free.
  - VectorE (DVE): two fused scalar_tensor_tensor ops, each doing
    clamp(max, -100) + multiply-by-gt + row-reduce in one instruction
    (A and B).  gt (int32) is consumed directly as the in1 operand.
  - GPSIMD: tensor_reduce of gt for s.
All engines stay below the DMA roofline (16.8 MB/core @ 358 GB/s ~ 47us).
Host gathers the [128, ntiles] partials from all 8 cores and finishes
the (tiny) all-reduce + final scalar arithmetic in float64.
"""

import numpy as np
from contextlib import ExitStack

import concourse.bass as bass
import concourse.bacc as bacc
import concourse.mybir as mybir
import concourse.tile as tile
from concourse.alu_op_type import AluOpType
from concourse.bass_utils import run_bass_kernel_spmd

N_TOTAL = 16777216
N_CORES = 8
PER_CORE = N_TOTAL // N_CORES   # 2097152
P = 128
FD = PER_CORE // P              # 16384 free elements per partition
TILE_FD = 2048
NT = FD // TILE_FD              # 8 tiles per core
LOG_CLAMP = -100.0

# Optional instrumentation knobs for a driver script (harness never sets them).
TRACE = False
LAST_RESULTS = None

_NC_CACHE = None


def _build():
    f32 = mybir.dt.float32
    i32 = mybir.dt.int32
    Ln = mybir.ActivationFunctionType.Ln

    nc = bacc.Bacc("TRN2")
    x_in = nc.declare_dram_parameter("x", [P, FD], f32, isOutput=False)
    g_in = nc.declare_dram_parameter("gt", [P, FD], i32, isOutput=False)
    outa = nc.declare_dram_parameter("outa", [P, NT], f32, isOutput=True)
    outb = nc.declare_dram_parameter("outb", [P, NT], f32, isOutput=True)
    outc = nc.declare_dram_parameter("outc", [P, NT], f32, isOutput=True)
    outs = nc.declare_dram_parameter("outs", [1, NT], i32, isOutput=True)

    with tile.TileContext(nc) as tc, ExitStack() as ctx:
        xp = ctx.enter_context(tc.tile_pool(name="xp", bufs=3))
        gp = ctx.enter_context(tc.tile_pool(name="gp", bufs=3))
        lp = ctx.enter_context(tc.tile_pool(name="lp", bufs=2))
        jp = ctx.enter_context(tc.tile_pool(name="jp", bufs=2))
        accp = ctx.enter_context(tc.tile_pool(name="accp", bufs=1))

        accA = accp.tile([P, NT], f32)
        accB = accp.tile([P, NT], f32)
        accC = accp.tile([P, NT], f32)
        accS = accp.tile([1, NT], i32)

        for i in range(NT):
            sl = bass.ts(i, TILE_FD)
            xt = xp.tile([P, TILE_FD], f32)
            gt_t = gp.tile([P, TILE_FD], i32)
            nc.sync.dma_start(xt[:], x_in[:, sl])
            nc.sync.dma_start(gt_t[:], g_in[:, sl])

            lnx = lp.tile([P, TILE_FD], f32, tag="lnx")
            ln1 = lp.tile([P, TILE_FD], f32, tag="ln1")
            nc.scalar.activation(lnx[:], xt[:], Ln)
            nc.scalar.activation(
                ln1[:], xt[:], Ln, bias=1.0, scale=-1.0,
                accum_out=accC[:, i : i + 1],
            )

            junk = jp.tile([P, TILE_FD], f32, tag="junk")
            nc.vector.scalar_tensor_tensor(
                junk[:], lnx[:], LOG_CLAMP, gt_t[:],
                AluOpType.max, AluOpType.mult,
                accum_out=accA[:, i : i + 1],
            )
            junk2 = jp.tile([P, TILE_FD], f32, tag="junk")
            nc.vector.scalar_tensor_tensor(
                junk2[:], ln1[:], LOG_CLAMP, gt_t[:],
                AluOpType.max, AluOpType.mult,
                accum_out=accB[:, i : i + 1],
            )

            with nc.allow_low_precision(reason="int32 count of 0/1 values is exact"):
                nc.gpsimd.tensor_reduce(
                    accS[:, i : i + 1], gt_t[:],
                    axis=mybir.AxisListType.XYZWC, op=AluOpType.add,
                )

        nc.sync.dma_start(outa[:], accA[:])
        nc.sync.dma_start(outb[:], accB[:])
        nc.sync.dma_start(outc[:], accC[:])
        nc.sync.dma_start(outs[:], accS[:])

    nc.compile()
    return nc


def get_nc():
    global _NC_CACHE
    if _NC_CACHE is None:
        _NC_CACHE = _build()
    return _NC_CACHE


def make_in_maps(x, gt):
    x = np.ascontiguousarray(np.asarray(x, dtype=np.float32).reshape(-1))
    gt = np.ascontiguousarray(np.asarray(gt, dtype=np.int32).reshape(-1))
    assert x.shape == (N_TOTAL,) and gt.shape == (N_TOTAL,)
    in_maps = []
    for c in range(N_CORES):
        sl = slice(c * PER_CORE, (c + 1) * PER_CORE)
        in_maps.append({
            "x": x[sl].reshape(P, FD),
            "gt": gt[sl].reshape(P, FD),
        })
    return in_maps


def combine(results):
    """All-reduce the per-core partial sums and finish the loss formula."""
    A = B = C = S = 0.0
    for r in results:
        A += r["outa"].astype(np.float64).sum()
        B += r["outb"].astype(np.float64).sum()
        C += r["outc"].astype(np.float64).sum()
        S += r["outs"].astype(np.float64).sum()
    n = float(N_TOTAL)
    result = -(A / (2.0 * S) + (C - B) / (2.0 * (n - S)))
    return np.array(result, dtype=np.float32)


def kernel(x, gt):
    global LAST_RESULTS
    nc = get_nc()
    in_maps = make_in_maps(x, gt)
    br = run_bass_kernel_spmd(nc, in_maps, list(range(N_CORES)))
    LAST_RESULTS = br
    return combine(br.results)


# revision 13
# speedup vs baseline: 1.1307x; 1.1307x over previous
"""Weighted-BCE loss kernel for Trainium2 (8 NeuronCores, SPMD data-parallel).

Reference math (torch-style BCELoss with class-balancing weights):
    n   = len(x), s = sum(gt)
    w0  = n / (2*(n-s)),  w1 = n / (2*s)
    L1  = max(log(x),     -100)
    L0  = max(log1p(-x),  -100)
    loss = mean( where(gt==0, w0, w1) * -(gt*L1 + (1-gt)*L0) )

The weights depend only on the GLOBAL positive count s, so the loss
decomposes into 4 global sums computed shard-locally:
    A = sum(gt * L1),  B = sum(gt * L0),  C = sum(L0),  s = sum(gt)
    loss = -( A/(2s) + (C-B)/(2(n-s)) )

Each core processes a 1/8 shard laid out [128 partitions, 16384 free]:
  - ScalarE (ACT): Ln(x), and Ln(1-x) via the free affine (scale=-1,
    bias=1); the second op's accum_out produces C for free.
  - VectorE (DVE): two fused scalar_tensor_tensor ops, each doing
    clamp(max, -100) + multiply-by-gt + row-reduce in one instruction
    (A and B).  gt (int32) is consumed directly as the in1 operand.
  - GPSIMD: tensor_reduce of gt for s.
All engines stay below the DMA roofline (16.8 MB/core @ 358 GB/s ~ 47us).
Host gathers the [128, ntiles] partials from all 8 cores and finishes
the (tiny) all-reduce + final scalar arithmetic in float64.
"""

import numpy as np
from contextlib import ExitStack

import concourse.bass as bass
import concourse.bacc as bacc
import concourse.mybir as mybir
import concourse.tile as tile
from concourse.alu_op_type import AluOpType
from concourse.bass_utils import run_bass_kernel_spmd

N_TOTAL = 16777216
N_CORES = 8
PER_CORE = N_TOTAL // N_CORES   # 2097152
P = 128
FD = PER_CORE // P              # 16384 free elements per partition
TILE_FD = 2048
NT = FD // TILE_FD              # 8 tiles per core
LOG_CLAMP = -100.0

# Optional instrumentation knobs for a driver script (harness never sets them).
TRACE = False
LAST_RESULTS = None

_NC_CACHE = None


def _build():
    f32 = mybir.dt.float32
    i32 = mybir.dt.int32
    Ln = mybir.ActivationFunctionType.Ln

    nc = bacc.Bacc("TRN2")
    x_in = nc.declare_dram_parameter("x", [P, FD], f32, isOutput=False)
    g_in = nc.declare_dram_parameter("gt", [P, FD], i32, isOutput=False)
    outa = nc.declare_dram_parameter("outa", [P, NT], f32, isOutput=True)
    outb = nc.declare_dram_parameter("outb", [P, NT], f32, isOutput=True)
    outc = nc.declare_dram_parameter("outc", [P, NT], f32, isOutput=True)
    outs = nc.declare_dram_parameter("outs", [P, NT], f32, isOutput=True)

    with tile.TileContext(nc) as tc, ExitStack() as ctx:
        xp = ctx.enter_context(tc.tile_pool(name="xp", bufs=3))
        gp = ctx.enter_context(tc.tile_pool(name="gp", bufs=3))
        lp = ctx.enter_context(tc.tile_pool(name="lp", bufs=2))
        jp = ctx.enter_context(tc.tile_pool(name="jp", bufs=2))
        accp = ctx.enter_context(tc.tile_pool(name="accp", bufs=1))

        accA = accp.tile([P, NT], f32)
        accB = accp.tile([P, NT], f32)
        accC = accp.tile([P, NT], f32)
        accS = accp.tile([P, NT], f32)

        for i in range(NT):
            sl = bass.ts(i, TILE_FD)
            xt = xp.tile([P, TILE_FD], f32)
            gt_t = gp.tile([P, TILE_FD], i32)
            # two HWDGE queues: x via SP(sync), gt via the ACT sequencer
            nc.sync.dma_start(xt[:], x_in[:, sl])
            nc.scalar.dma_start(gt_t[:], g_in[:, sl])

            lnx = lp.tile([P, TILE_FD], f32, tag="lnx")
            ln1 = lp.tile([P, TILE_FD], f32, tag="ln1")
            nc.scalar.activation(lnx[:], xt[:], Ln)
            nc.scalar.activation(
                ln1[:], xt[:], Ln, bias=1.0, scale=-1.0,
                accum_out=accC[:, i : i + 1],
            )

            junk = jp.tile([P, TILE_FD], f32, tag="junk")
            nc.vector.scalar_tensor_tensor(
                junk[:], lnx[:], LOG_CLAMP, gt_t[:],
                AluOpType.max, AluOpType.mult,
                accum_out=accA[:, i : i + 1],
            )
            junk2 = jp.tile([P, TILE_FD], f32, tag="junk")
            nc.vector.scalar_tensor_tensor(
                junk2[:], ln1[:], LOG_CLAMP, gt_t[:],
                AluOpType.max, AluOpType.mult,
                accum_out=accB[:, i : i + 1],
            )
            # s = sum(gt): ACT copy (i32 -> f32) with a free row-sum accumulate
            junk3 = jp.tile([P, TILE_FD], f32, tag="junk3")
            nc.scalar.activation(
                junk3[:], gt_t[:], mybir.ActivationFunctionType.Copy,
                accum_out=accS[:, i : i + 1],
            )

        nc.sync.dma_start(outa[:], accA[:])
        nc.sync.dma_start(outb[:], accB[:])
        nc.sync.dma_start(outc[:], accC[:])
        nc.sync.dma_start(outs[:], accS[:])

    nc.compile()
    return nc


def get_nc():
    global _NC_CACHE
    if _NC_CACHE is None:
        _NC_CACHE = _build()
    return _NC_CACHE


def make_in_maps(x, gt):
    x = np.ascontiguousarray(np.asarray(x, dtype=np.float32).reshape(-1))
    gt = np.ascontiguousarray(np.asarray(gt, dtype=np.int32).reshape(-1))
    assert x.shape == (N_TOTAL,) and gt.shape == (N_TOTAL,)
    in_maps = []
    for c in range(N_CORES):
        sl = slice(c * PER_CORE, (c + 1) * PER_CORE)
        in_maps.append({
            "x": x[sl].reshape(P, FD),
            "gt": gt[sl].reshape(P, FD),
        })
    return in_maps


def combine(results):
    """All-reduce the per-core partial sums and finish the loss formula."""
    A = B = C = S = 0.0
    for r in results:
        A += r["outa"].astype(np.float64).sum()
        B += r["outb"].astype(np.float64).sum()
        C += r["outc"].astype(np.float64).sum()
        S += r["outs"].astype(np.float64).sum()
    n = float(N_TOTAL)
    result = -(A / (2.0 * S) + (C - B) / (2.0 * (n - S)))
    return np.array(result, dtype=np.float32)


def kernel(x, gt):
    global LAST_RESULTS
    nc = get_nc()
    in_maps = make_in_maps(x, gt)
    br = run_bass_kernel_spmd(nc, in_maps, list(range(N_CORES)))
    LAST_RESULTS = br
    return combine(br.results)


# revision 17
# speedup vs baseline: 1.1391x; 1.0075x over previous
"""Weighted-BCE loss kernel for Trainium2 (8 NeuronCores, SPMD data-parallel).

Reference math (torch-style BCELoss with class-balancing weights):
    n   = len(x), s = sum(gt)
    w0  = n / (2*(n-s)),  w1 = n / (2*s)
    L1  = max(log(x),     -100)
    L0  = max(log1p(-x),  -100)
    loss = mean( where(gt==0, w0, w1) * -(gt*L1 + (1-gt)*L0) )

The weights depend only on the GLOBAL positive count s, so the loss
decomposes into 4 global sums computed shard-locally:
    A = sum(gt * L1),  B = sum(gt * L0),  C = sum(L0),  s = sum(gt)
    loss = -( A/(2s) + (C-B)/(2(n-s)) )

Each core processes a 1/8 shard laid out [128 partitions, 16384 free]:
  - ScalarE (ACT): Ln(x), and Ln(1-x) via the free affine (scale=-1,
    bias=1); the second op's accum_out produces C for free.
  - VectorE (DVE): two fused scalar_tensor_tensor ops, each doing
    clamp(max, -100) + multiply-by-gt + row-reduce in one instruction
    (A and B).  gt (int32) is consumed directly as the in1 operand.
  - GPSIMD: tensor_reduce of gt for s.
All engines stay below the DMA roofline (16.8 MB/core @ 358 GB/s ~ 47us).
Host gathers the [128, ntiles] partials from all 8 cores and finishes
the (tiny) all-reduce + final scalar arithmetic in float64.
"""

import numpy as np
from contextlib import ExitStack

import concourse.bass as bass
import concourse.bacc as bacc
import concourse.mybir as mybir
import concourse.tile as tile
from concourse.alu_op_type import AluOpType
from concourse.bass_utils import run_bass_kernel_spmd

N_TOTAL = 16777216
N_CORES = 8
PER_CORE = N_TOTAL // N_CORES   # 2097152
P = 128
FD = PER_CORE // P              # 16384 free elements per partition
TILE_FD = 2048
NT = FD // TILE_FD              # 8 tiles per core
LOG_CLAMP = -100.0

# Optional instrumentation knobs for a driver script (harness never sets them).
TRACE = False
LAST_RESULTS = None

_NC_CACHE = None


def _build():
    f32 = mybir.dt.float32
    i32 = mybir.dt.int32
    Ln = mybir.ActivationFunctionType.Ln

    nc = bacc.Bacc("TRN2")
    x_in = nc.declare_dram_parameter("x", [P, FD], f32, isOutput=False)
    g_in = nc.declare_dram_parameter("gt", [P, FD], i32, isOutput=False)
    outa = nc.declare_dram_parameter("outa", [P, NT], f32, isOutput=True)
    outb = nc.declare_dram_parameter("outb", [P, NT], f32, isOutput=True)
    outc = nc.declare_dram_parameter("outc", [P, NT], f32, isOutput=True)
    outs = nc.declare_dram_parameter("outs", [1, 1], i32, isOutput=True)

    with tile.TileContext(nc) as tc, ExitStack() as ctx:
        xp = ctx.enter_context(tc.tile_pool(name="xp", bufs=3))
        gp = ctx.enter_context(tc.tile_pool(name="gp", bufs=3))
        lp = ctx.enter_context(tc.tile_pool(name="lp", bufs=2))
        jp = ctx.enter_context(tc.tile_pool(name="jp", bufs=2))
        accp = ctx.enter_context(tc.tile_pool(name="accp", bufs=1))

        accA = accp.tile([P, NT], f32)
        accB = accp.tile([P, NT], f32)
        accC = accp.tile([P, NT], f32)
        accS = accp.tile([1, 1], i32)
        gsum = accp.tile([P, TILE_FD], i32)
        nc.gpsimd.memset(gsum[:], 0)

        for i in range(NT):
            sl = bass.ts(i, TILE_FD)
            xt = xp.tile([P, TILE_FD], f32)
            gt_t = gp.tile([P, TILE_FD], i32)
            # two HWDGE queues: x via SP(sync), gt via the ACT sequencer
            nc.sync.dma_start(xt[:], x_in[:, sl])
            nc.scalar.dma_start(gt_t[:], g_in[:, sl])

            lnx = lp.tile([P, TILE_FD], f32, tag="lnx")
            ln1 = lp.tile([P, TILE_FD], f32, tag="ln1")
            nc.scalar.activation(lnx[:], xt[:], Ln)
            nc.scalar.activation(
                ln1[:], xt[:], Ln, bias=1.0, scale=-1.0,
                accum_out=accC[:, i : i + 1],
            )

            junk = jp.tile([P, TILE_FD], f32, tag="junk")
            nc.vector.scalar_tensor_tensor(
                junk[:], lnx[:], LOG_CLAMP, gt_t[:],
                AluOpType.max, AluOpType.mult,
                accum_out=accA[:, i : i + 1],
            )
            junk2 = jp.tile([P, TILE_FD], f32, tag="junk")
            nc.vector.scalar_tensor_tensor(
                junk2[:], ln1[:], LOG_CLAMP, gt_t[:],
                AluOpType.max, AluOpType.mult,
                accum_out=accB[:, i : i + 1],
            )
            # s = sum(gt): integer accumulate on the otherwise-idle GPSIMD
            nc.gpsimd.tensor_add(gsum[:], gsum[:], gt_t[:])

        with nc.allow_low_precision(reason="int32 count of 0/1 values is exact"):
            nc.gpsimd.tensor_reduce(
                accS[:], gsum[:],
                axis=mybir.AxisListType.XYZWC, op=AluOpType.add,
            )
        nc.sync.dma_start(outa[:], accA[:])
        nc.sync.dma_start(outb[:], accB[:])
        nc.sync.dma_start(outc[:], accC[:])
        nc.sync.dma_start(outs[:], accS[:])

    nc.compile()
    return nc


def get_nc():
    global _NC_CACHE
    if _NC_CACHE is None:
        _NC_CACHE = _build()
    return _NC_CACHE


def make_in_maps(x, gt):
    x = np.ascontiguousarray(np.asarray(x, dtype=np.float32).reshape(-1))
    gt = np.ascontiguousarray(np.asarray(gt, dtype=np.int32).reshape(-1))
    assert x.shape == (N_TOTAL,) and gt.shape == (N_TOTAL,)
    in_maps = []
    for c in range(N_CORES):
        sl = slice(c * PER_CORE, (c + 1) * PER_CORE)
        in_maps.append({
            "x": x[sl].reshape(P, FD),
            "gt": gt[sl].reshape(P, FD),
        })
    return in_maps


def combine(results):
    """All-reduce the per-core partial sums and finish the loss formula."""
    A = B = C = S = 0.0
    for r in results:
        A += r["outa"].astype(np.float64).sum()
        B += r["outb"].astype(np.float64).sum()
        C += r["outc"].astype(np.float64).sum()
        S += r["outs"].astype(np.float64).sum()
    n = float(N_TOTAL)
    result = -(A / (2.0 * S) + (C - B) / (2.0 * (n - S)))
    return np.array(result, dtype=np.float32)


def kernel(x, gt):
    global LAST_RESULTS
    nc = get_nc()
    in_maps = make_in_maps(x, gt)
    br = run_bass_kernel_spmd(nc, in_maps, list(range(N_CORES)))
    LAST_RESULTS = br
    return combine(br.results)


# revision 18
# speedup vs baseline: 1.2548x; 1.1015x over previous
"""Weighted-BCE loss kernel for Trainium2 (8 NeuronCores, SPMD data-parallel).

Reference math (torch-style BCELoss with class-balancing weights):
    n   = len(x), s = sum(gt)
    w0  = n / (2*(n-s)),  w1 = n / (2*s)
    L1  = max(log(x),     -100)
    L0  = max(log1p(-x),  -100)
    loss = mean( where(gt==0, w0, w1) * -(gt*L1 + (1-gt)*L0) )

The weights depend only on the GLOBAL positive count s, so the loss
decomposes into 4 global sums computed shard-locally:
    A = sum(gt * L1),  B = sum(gt * L0),  C = sum(L0),  s = sum(gt)
    loss = -( A/(2s) + (C-B)/(2(n-s)) )

Each core processes a 1/8 shard laid out [128 partitions, 16384 free]:
  - ScalarE (ACT): Ln(x), and Ln(1-x) via the free affine (scale=-1,
    bias=1); the second op's accum_out produces C for free.
  - VectorE (DVE): two fused scalar_tensor_tensor ops, each doing
    clamp(max, -100) + multiply-by-gt + row-reduce in one instruction
    (A and B).  gt (int32) is consumed directly as the in1 operand.
  - GPSIMD: tensor_reduce of gt for s.
All engines stay below the DMA roofline (16.8 MB/core @ 358 GB/s ~ 47us).
Host gathers the [128, ntiles] partials from all 8 cores and finishes
the (tiny) all-reduce + final scalar arithmetic in float64.
"""

import numpy as np
from contextlib import ExitStack

import concourse.bass as bass
import concourse.bacc as bacc
import concourse.mybir as mybir
import concourse.tile as tile
from concourse.alu_op_type import AluOpType
from concourse.bass_utils import run_bass_kernel_spmd

N_TOTAL = 16777216
N_CORES = 8
PER_CORE = N_TOTAL // N_CORES   # 2097152
P = 128
FD = PER_CORE // P              # 16384 free elements per partition
TILE_FD = 4096
NT = FD // TILE_FD              # 8 tiles per core
LOG_CLAMP = -100.0

# Optional instrumentation knobs for a driver script (harness never sets them).
TRACE = False
LAST_RESULTS = None

_NC_CACHE = None


def _build():
    f32 = mybir.dt.float32
    i32 = mybir.dt.int32
    Ln = mybir.ActivationFunctionType.Ln

    nc = bacc.Bacc("TRN2")
    x_in = nc.declare_dram_parameter("x", [P, FD], f32, isOutput=False)
    g_in = nc.declare_dram_parameter("gt", [P, FD], i32, isOutput=False)
    outa = nc.declare_dram_parameter("outa", [P, NT], f32, isOutput=True)
    outb = nc.declare_dram_parameter("outb", [P, NT], f32, isOutput=True)
    outc = nc.declare_dram_parameter("outc", [P, NT], f32, isOutput=True)
    outs = nc.declare_dram_parameter("outs", [P, NT], f32, isOutput=True)

    with tile.TileContext(nc) as tc, ExitStack() as ctx:
        xp = ctx.enter_context(tc.tile_pool(name="xp", bufs=3))
        gp = ctx.enter_context(tc.tile_pool(name="gp", bufs=3))
        lp = ctx.enter_context(tc.tile_pool(name="lp", bufs=2))
        jp = ctx.enter_context(tc.tile_pool(name="jp", bufs=1))
        accp = ctx.enter_context(tc.tile_pool(name="accp", bufs=1))

        accA = accp.tile([P, NT], f32)
        accB = accp.tile([P, NT], f32)
        accC = accp.tile([P, NT], f32)
        accS = accp.tile([P, NT], f32)

        for i in range(NT):
            sl = bass.ts(i, TILE_FD)
            xt = xp.tile([P, TILE_FD], f32)
            gt_t = gp.tile([P, TILE_FD], i32)
            # two HWDGE queues: x via SP(sync), gt via the ACT sequencer
            nc.sync.dma_start(xt[:], x_in[:, sl])
            nc.scalar.dma_start(gt_t[:], g_in[:, sl])

            lnx = lp.tile([P, TILE_FD], f32, tag="lnx")
            ln1 = lp.tile([P, TILE_FD], f32, tag="ln1")
            nc.scalar.activation(lnx[:], xt[:], Ln)
            nc.scalar.activation(
                ln1[:], xt[:], Ln, bias=1.0, scale=-1.0,
                accum_out=accC[:, i : i + 1],
            )

            junk = jp.tile([P, TILE_FD], f32, tag="junk")
            nc.vector.scalar_tensor_tensor(
                junk[:], lnx[:], LOG_CLAMP, gt_t[:],
                AluOpType.max, AluOpType.mult,
                accum_out=accA[:, i : i + 1],
            )
            junk2 = jp.tile([P, TILE_FD], f32, tag="junk")
            nc.vector.scalar_tensor_tensor(
                junk2[:], ln1[:], LOG_CLAMP, gt_t[:],
                AluOpType.max, AluOpType.mult,
                accum_out=accB[:, i : i + 1],
            )
            # s = sum(gt): ACT copy (i32 -> f32) with a free row-sum accumulate
            junk3 = jp.tile([P, TILE_FD], f32, tag="junk3")
            nc.scalar.activation(
                junk3[:], gt_t[:], mybir.ActivationFunctionType.Copy,
                accum_out=accS[:, i : i + 1],
            )

        nc.sync.dma_start(outa[:], accA[:])
        nc.sync.dma_start(outb[:], accB[:])
        nc.sync.dma_start(outc[:], accC[:])
        nc.sync.dma_start(outs[:], accS[:])

    nc.compile()
    return nc


def get_nc():
    global _NC_CACHE
    if _NC_CACHE is None:
        _NC_CACHE = _build()
    return _NC_CACHE


def make_in_maps(x, gt):
    x = np.ascontiguousarray(np.asarray(x, dtype=np.float32).reshape(-1))
    gt = np.ascontiguousarray(np.asarray(gt, dtype=np.int32).reshape(-1))
    assert x.shape == (N_TOTAL,) and gt.shape == (N_TOTAL,)
    in_maps = []
    for c in range(N_CORES):
        sl = slice(c * PER_CORE, (c + 1) * PER_CORE)
        in_maps.append({
            "x": x[sl].reshape(P, FD),
            "gt": gt[sl].reshape(P, FD),
        })
    return in_maps


def combine(results):
    """All-reduce the per-core partial sums and finish the loss formula."""
    A = B = C = S = 0.0
    for r in results:
        A += r["outa"].astype(np.float64).sum()
        B += r["outb"].astype(np.float64).sum()
        C += r["outc"].astype(np.float64).sum()
        S += r["outs"].astype(np.float64).sum()
    n = float(N_TOTAL)
    result = -(A / (2.0 * S) + (C - B) / (2.0 * (n - S)))
    return np.array(result, dtype=np.float32)


def kernel(x, gt):
    global LAST_RESULTS
    nc = get_nc()
    in_maps = make_in_maps(x, gt)
    br = run_bass_kernel_spmd(nc, in_maps, list(range(N_CORES)))
    LAST_RESULTS = br
    return combine(br.results)
